# revision 25
# baseline (speedup 1.0000x reference)
"""DeepSeek sparse attention (single-query, MQA low-rank KV) on 8 trn2 cores.

Folded-indexer design (data-parallel: batch b -> core b):

The low-rank structure lets every per-token device matmul collapse into a
small set of per-batch vectors computed once on host:
  scores[s] = x[s] . w_eff          w_eff = Wdk @ q_idx           [D]
  logit[h,s] = x[s] . w_l[:,h]      w_l = Wdk @ (Wk_up_h @ q_h)   [D, 16]
so launch 1 is ONE fused fp8 DoubleRow matmul stream over x^T (the full
16.8 MB shard read once, memory-bound): stationary = [16 fp8 logit cols |
16 fp8 residual cols (64x error compensation) | 1 score col], out =
[33, 8192] f32.  Host: top-k with exact band rescore (bit-exact vs the
reference's fp8 indexer), softmax over gathered logits.
Launch 2: attention value aggregation ax[d,h] = sum_k attn[k,h]*x_sel[k,d]
(bf16, 8 MiB stream).  Host tail: tiny per-token GEMMs (ax @ Wdv etc.,
~0.5 GFLOP total, same class as the baseline's host-side q projection).

Shapes hardcoded: B=8, S=8192, D=2048, H=16, dh=128, L=512, k=2048.
"""
import numpy as np
import ml_dtypes

import concourse.bacc as bacc
import concourse.tile as tile
import concourse.mybir as mybir
from concourse.bass_utils import run_bass_kernel_spmd

BF16 = ml_dtypes.bfloat16
F8 = ml_dtypes.float8_e4m3fn
dt = mybir.dt
DR = mybir.MatmulPerfMode.DoubleRow

B, S, D = 8, 8192, 2048
H, DH, L = 16, 128, 512
TOPK = 2048
MARGIN = 768
NCORES = 8
RSQ = float(1.0 / np.sqrt(np.float32(DH)))
RESID_SCALE = 64.0

ND = D // 128            # 16 d-chunks
NP = ND // 2             # 8 DoubleRow chunk pairs

_STATE = {}


# ---------------------------------------------------------------- launch 1
def _build_l1():
    """Fused scores+logits+residuals over the full sequence.

    PSUM holds [33, 4096] f32 per pass (8 banks x [33, 512]); two passes
    cover S=8192. Pass A overlaps the x DMA stream; pass B runs from
    SBUF-resident x right after, bank-by-bank behind pass A's drains."""
    nc = bacc.Bacc("TRN2", target_bir_lowering=False, debug=False,
                   num_devices=NCORES)
    xT8 = nc.dram_tensor("xT8", [D, S], dt.float8e4, kind="ExternalInput").ap()
    wfl = nc.dram_tensor("wfl", [128, ND, 48], dt.float8e4,
                         kind="ExternalInput").ap()
    sl = nc.dram_tensor("sl", [33, S], dt.bfloat16, kind="ExternalOutput").ap()

    with tile.TileContext(nc) as tc:
        with (
            tc.tile_pool(name="sb", bufs=1) as sb,
            tc.tile_pool(name="ps", bufs=1, space="PSUM") as ps,
        ):
            ws = sb.tile([128, ND, 48], dt.float8e4)
            nc.scalar.dma_start(ws[:], wfl)
            # phased stream: column-half A (s 0-4095) of every chunk first,
            # then column-half B — pass A's whole contraction completes and
            # drains mid-stream, pass B mirrors it under the second half;
            # only pass B's final pair trails the last byte.
            xs = sb.tile([128, ND, S], dt.float8e4)
            for c in range(ND):
                nc.sync.dma_start(xs[:, c:c + 1, 0:4096],
                                  xT8[c * 128:(c + 1) * 128, 0:4096])
            for c in range(ND - 2):
                nc.sync.dma_start(xs[:, c:c + 1, 4096:8192],
                                  xT8[c * 128:(c + 1) * 128, 4096:8192])
            # last two B-halves in 1024-col pieces so each slab's final
            # matmul starts on its own slice
            for i in range(4):
                for c in (ND - 2, ND - 1):
                    nc.sync.dma_start(
                        xs[:, c:c + 1, 4096 + i * 1024:4096 + (i + 1) * 1024],
                        xT8[c * 128:(c + 1) * 128,
                            4096 + i * 1024:4096 + (i + 1) * 1024])
            stg = sb.tile([33, S], dt.bfloat16)

            banks = [ps.tile([128, 512], dt.float32, name=f"bk{i}")
                     for i in range(8)]
            # pass A rides the A-half stream
            for p in range(NP):
                for i in range(8):
                    nc.tensor.matmul(
                        banks[i][0:33, :],
                        ws[:, 2 * p:2 * p + 2, 0:33],
                        xs[:, 2 * p:2 * p + 2, i * 512:(i + 1) * 512],
                        start=(p == 0), stop=(p == NP - 1), perf_mode=DR)
            for i in range(8):
                eng = nc.vector.tensor_copy if i % 2 == 0 else nc.scalar.copy
                eng(stg[:, i * 512:(i + 1) * 512], banks[i][0:33, :])
            nc.sync.dma_start(sl[:, 0:4096], stg[:, 0:4096])
            # pass B rides the B-half stream (same banks, fresh groups)
            for p in range(NP - 1):
                for i in range(8):
                    nc.tensor.matmul(
                        banks[i][0:33, :],
                        ws[:, 2 * p:2 * p + 2, 0:33],
                        xs[:, 2 * p:2 * p + 2,
                           4096 + i * 512:4096 + (i + 1) * 512],
                        start=(p == 0), stop=False, perf_mode=DR)
            p = NP - 1
            for i in range(8):
                nc.tensor.matmul(
                    banks[i][0:33, :],
                    ws[:, 2 * p:2 * p + 2, 0:33],
                    xs[:, 2 * p:2 * p + 2,
                       4096 + i * 512:4096 + (i + 1) * 512],
                    start=False, stop=True, perf_mode=DR)
                eng = nc.vector.tensor_copy if i % 2 == 1 else nc.scalar.copy
                eng(stg[:, 4096 + i * 512:4096 + (i + 1) * 512],
                    banks[i][0:33, :])
                if i in (1, 3, 5):
                    nc.sync.dma_start(
                        sl[:, 4096 + (i - 1) * 512:4096 + (i + 1) * 512],
                        stg[:, 4096 + (i - 1) * 512:4096 + (i + 1) * 512])
                elif i == 6:
                    nc.sync.dma_start(sl[:, 7168:7680], stg[:, 7168:7680])
                elif i == 7:
                    nc.sync.dma_start(sl[:, 7680:8192], stg[:, 7680:8192])
    nc.compile()
    return nc


# ---------------------------------------------------------------- launch 2
def _build_l2():
    """ax[h, d] = sum_k attn[k, h] * x_sel[k, d] in bf16.

    attnT chunks [128k, 16] are STATIONARY (M=16), x_sel streams as the
    moving operand -> out [16, D] f32 in 4 PSUM banks; contraction over
    the 16 k-chunks overlaps the x_sel DMA stream; one row-contiguous
    output DMA."""
    nc = bacc.Bacc("TRN2", target_bir_lowering=False, debug=False,
                   num_devices=NCORES)
    NK = TOPK // 128     # 16 k-chunks
    xsel = nc.dram_tensor("xsel", [TOPK, D], dt.bfloat16,
                          kind="ExternalInput").ap()
    att = nc.dram_tensor("att", [128, NK, H], dt.bfloat16,
                         kind="ExternalInput").ap()
    axh = nc.dram_tensor("axh", [H, D], dt.float32, kind="ExternalOutput").ap()

    with tile.TileContext(nc) as tc:
        with (
            tc.tile_pool(name="sb", bufs=1) as sb,
            tc.tile_pool(name="ps", bufs=1, space="PSUM") as ps,
        ):
            ats = sb.tile([128, NK, H], dt.bfloat16)
            nc.scalar.dma_start(ats[:], att)
            xss = sb.tile([128, NK, D], dt.bfloat16)
            for kc in range(NK - 2):
                nc.sync.dma_start(xss[:, kc:kc + 1, :],
                                  xsel[kc * 128:(kc + 1) * 128, :])
            for nb in range(2):
                nc.sync.dma_start(
                    xss[:, NK - 2:NK - 1, nb * 1024:(nb + 1) * 1024],
                    xsel[(NK - 2) * 128:(NK - 1) * 128,
                         nb * 1024:(nb + 1) * 1024])
            # last k-chunk in 512-col pieces: each nb-matmul starts on its
            # own slice instead of waiting for the full chunk
            for nb in range(4):
                nc.sync.dma_start(
                    xss[:, NK - 1:NK, nb * 512:(nb + 1) * 512],
                    xsel[(NK - 1) * 128:NK * 128, nb * 512:(nb + 1) * 512])
            banks = [ps.tile([128, 512], dt.float32, name=f"ab{i}")
                     for i in range(4)]
            stg = sb.tile([H, D], dt.float32)
            for kc in range(NK - 1):
                for nb in range(4):
                    nc.tensor.matmul(
                        banks[nb][0:H, :],
                        ats[:, kc:kc + 1, :],
                        xss[:, kc:kc + 1, nb * 512:(nb + 1) * 512],
                        start=(kc == 0), stop=False)
            # final k-chunk: drain each bank right behind its last matmul
            kc = NK - 1
            for nb in range(4):
                nc.tensor.matmul(
                    banks[nb][0:H, :],
                    ats[:, kc:kc + 1, :],
                    xss[:, kc:kc + 1, nb * 512:(nb + 1) * 512],
                    start=False, stop=True)
                eng = nc.vector.tensor_copy if nb % 2 == 0 else nc.scalar.copy
                eng(stg[:, nb * 512:(nb + 1) * 512], banks[nb][0:H, :])
                if nb % 2 == 1:
                    nc.sync.dma_start(
                        axh[:, (nb - 1) * 512:(nb + 1) * 512],
                        stg[:, (nb - 1) * 512:(nb + 1) * 512])
    nc.compile()
    return nc


# ---------------------------------------------------------------- timing
def model_time(nc):
    """Cost-model (TimelineSim) estimate in ns for one core."""
    from concourse.timeline_sim import TimelineSim
    return TimelineSim(nc).simulate()


def _run_spmd_retry(nc, in_maps, cores, trace=False):
    """One retry: a previously crashed process can leave the device in a
    transient NRT_EXEC_UNIT_UNRECOVERABLE state that clears on re-run."""
    try:
        return run_bass_kernel_spmd(nc, in_maps, cores, trace=trace)
    except Exception:
        import time as _t
        _t.sleep(2.0)
        return run_bass_kernel_spmd(nc, in_maps, cores, trace=trace)


def _q8(a):
    return np.asarray(a, np.float32).astype(F8).astype(np.float32)


def kernel(**inputs):
    import jax
    import jax.numpy as jnp
    cpu = jax.devices("cpu")[0]

    x = np.ascontiguousarray(np.asarray(inputs["x"], dtype=np.float32))
    Wq = np.asarray(inputs["Wq"], dtype=np.float32)
    bq = np.asarray(inputs["bq"], dtype=np.float32)
    Wkv_down = np.asarray(inputs["Wkv_down"], dtype=np.float32)
    bkv_down = np.asarray(inputs["bkv_down"], dtype=np.float32)
    Wq_down = np.asarray(inputs["Wq_down"], dtype=np.float32)
    bq_down = np.asarray(inputs["bq_down"], dtype=np.float32)
    Wkv_up = np.asarray(inputs["Wkv_up"], dtype=np.float32)
    bkv_up = np.asarray(inputs["bkv_up"], dtype=np.float32)
    Wout = np.asarray(inputs["Wout"], dtype=np.float32)
    bout = np.asarray(inputs["bout"], dtype=np.float32)
    k = int(np.asarray(inputs["top_k"]))
    assert k == TOPK, f"kernel hardcoded for top_k={TOPK}, got {k}"

    Wdk = Wkv_down[:, :L]
    bkd = bkv_down[:L]
    Wdv = Wkv_down[:, L:]
    bvd = bkv_down[L:]
    Wk_up_h = Wkv_up[:, :D].reshape(L, H, DH)
    Wv_up_h = Wkv_up[:, D:].reshape(L, H, DH)
    bk_up_h = bkv_up[:D].reshape(H, DH)
    bv_up_h = bkv_up[D:].reshape(H, DH)

    if "l1" not in _STATE:
        _STATE["l1"] = _build_l1()
    if "l2" not in _STATE:
        _STATE["l2"] = _build_l2()

    q_last = x[:, -1, :]                                   # [B, D]
    with jax.default_device(cpu):
        # bit-exact replication of the reference's fp8 indexer query + q
        q_idx = np.asarray(
            jnp.asarray(_q8(q_last)) @ jnp.asarray(_q8(Wq_down))) \
            + _q8(bq_down)                                 # [B, L]
        q = (np.asarray(jnp.asarray(q_last) @ jnp.asarray(Wq)) + bq) \
            .reshape(B, H, DH)

    # folded per-batch vectors (host f32, exact linear algebra)
    v_lh = np.einsum('lhd,bhd->blh', Wk_up_h, q)           # [B, L, H]
    w_l = np.einsum('dl,blh->bdh', Wdk, v_lh)              # [B, D, H]
    c_l = np.einsum('l,blh->bh', bkd, v_lh) \
        + np.einsum('hd,bhd->bh', bk_up_h, q)              # [B, H]
    w_eff = q_idx @ Wdk.T                                  # [B, D]
    c_s = q_idx @ bkd                                      # [B]

    # ---------------- launch 1: fused noisy scores + logits over full S
    in1 = []
    for c in range(NCORES):
        w8 = w_l[c].astype(F8).astype(np.float32)          # [D, H]
        r8 = ((w_l[c] - w8) * RESID_SCALE).astype(F8)      # [D, H] fp8
        wfl = np.zeros((D, 48), F8)
        wfl[:, 0:16] = w8.astype(F8)
        wfl[:, 16:32] = r8
        wfl[:, 32] = w_eff[c].astype(F8)
        in1.append({
            "xT8": np.ascontiguousarray(x[c].T).astype(F8),
            "wfl": np.ascontiguousarray(
                wfl.reshape(ND, 128, 48).transpose(1, 0, 2)),
        })
    r1 = _run_spmd_retry(_STATE["l1"], in1, list(range(NCORES)))
    sl = np.stack([r1.results[c]["sl"] for c in range(NCORES)]) \
        .astype(np.float32)                                # [B, 33, S]

    # ---------------- host: exact top-k set via band rescore (bit-exact)
    sel_all = []
    logit_all = []
    with jax.default_device(cpu):
        jWdk = jnp.asarray(Wdk)
        jbkd = jnp.asarray(bkd)
        for b in range(B):
            s_noisy = sl[b, 32] + c_s[b]
            order = np.argsort(-np.maximum(s_noisy, 0.0), kind="stable")
            certain = order[:k - MARGIN]
            band = order[k - MARGIN:k + MARGIN]
            Kb = jnp.asarray(x[b][band]) @ jWdk + jbkd
            sb = np.asarray(jnp.einsum(
                "l,sl->s", jnp.asarray(q_idx[b]),
                Kb.astype(jnp.float8_e4m3fn).astype(jnp.float32)))
            sb = np.maximum(sb, 0.0)
            pick = band[np.argsort(-sb, kind="stable")[:k - len(certain)]]
            sel = np.concatenate([certain, pick])
            sel_all.append(sel)
            lg = (sl[b, 0:16][:, sel].T + sl[b, 16:32][:, sel].T / RESID_SCALE
                  + c_l[b][None, :]) * RSQ                 # [k, H]
            lg -= lg.max(axis=0, keepdims=True)
            e = np.exp(lg)
            logit_all.append(e / e.sum(axis=0, keepdims=True))

    # ---------------- launch 2: attention value aggregation
    NK = TOPK // 128
    in2 = []
    for c in range(NCORES):
        attT = logit_all[c].astype(BF16)                   # [k, H]
        in2.append({
            "xsel": x[c][sel_all[c]].astype(BF16),
            "att": np.ascontiguousarray(
                attT.reshape(NK, 128, H).transpose(1, 0, 2)),
        })
    r2 = _run_spmd_retry(_STATE["l2"], in2, list(range(NCORES)))
    ax = np.stack([r2.results[c]["axh"] for c in range(NCORES)])  # [B,H,D]

    # ---------------- host tail: tiny per-token projections (f32)
    o_lat = np.einsum('bhd,dl->bhl', ax, Wdv) + bvd        # [B, H, L]
    o = np.einsum('bhl,lhd->bhd', o_lat, Wv_up_h) + bv_up_h
    out = o.reshape(B, D) @ Wout + bout
    return out.astype(np.float32)


# revision 26
# speedup vs baseline: 1.0075x; 1.0075x over previous
"""DeepSeek sparse attention (single-query, MQA low-rank KV) on 8 trn2 cores.

Folded-indexer design (data-parallel: batch b -> core b):

The low-rank structure lets every per-token device matmul collapse into a
small set of per-batch vectors computed once on host:
  scores[s] = x[s] . w_eff          w_eff = Wdk @ q_idx           [D]
  logit[h,s] = x[s] . w_l[:,h]      w_l = Wdk @ (Wk_up_h @ q_h)   [D, 16]
so launch 1 is ONE fused fp8 DoubleRow matmul stream over x^T (the full
16.8 MB shard read once, memory-bound): stationary = [16 fp8 logit cols |
16 fp8 residual cols (64x error compensation) | 1 score col], out =
[33, 8192] f32.  Host: top-k with exact band rescore (bit-exact vs the
reference's fp8 indexer), softmax over gathered logits.
Launch 2: attention value aggregation ax[d,h] = sum_k attn[k,h]*x_sel[k,d]
(bf16, 8 MiB stream).  Host tail: tiny per-token GEMMs (ax @ Wdv etc.,
~0.5 GFLOP total, same class as the baseline's host-side q projection).

Shapes hardcoded: B=8, S=8192, D=2048, H=16, dh=128, L=512, k=2048.
"""
import numpy as np
import ml_dtypes

import concourse.bacc as bacc
import concourse.tile as tile
import concourse.mybir as mybir
from concourse.bass_utils import run_bass_kernel_spmd

BF16 = ml_dtypes.bfloat16
F8 = ml_dtypes.float8_e4m3fn
dt = mybir.dt
DR = mybir.MatmulPerfMode.DoubleRow

B, S, D = 8, 8192, 2048
H, DH, L = 16, 128, 512
TOPK = 2048
MARGIN = 768
NCORES = 8
RSQ = float(1.0 / np.sqrt(np.float32(DH)))
RESID_SCALE = 64.0

ND = D // 128            # 16 d-chunks
NP = ND // 2             # 8 DoubleRow chunk pairs

_STATE = {}


# ---------------------------------------------------------------- launch 1
def _build_l1():
    """Fused scores+logits+residuals over the full sequence.

    PSUM holds [33, 4096] f32 per pass (8 banks x [33, 512]); two passes
    cover S=8192. Pass A overlaps the x DMA stream; pass B runs from
    SBUF-resident x right after, bank-by-bank behind pass A's drains."""
    nc = bacc.Bacc("TRN2", target_bir_lowering=False, debug=False,
                   num_devices=NCORES)
    xT8 = nc.dram_tensor("xT8", [D, S], dt.float8e4, kind="ExternalInput").ap()
    wfl = nc.dram_tensor("wfl", [128, ND, 48], dt.float8e4,
                         kind="ExternalInput").ap()
    sl = nc.dram_tensor("sl", [33, S], dt.bfloat16, kind="ExternalOutput").ap()

    with tile.TileContext(nc) as tc:
        with (
            tc.tile_pool(name="sb", bufs=1) as sb,
            tc.tile_pool(name="ps", bufs=1, space="PSUM") as ps,
        ):
            ws = sb.tile([128, ND, 48], dt.float8e4)
            nc.scalar.dma_start(ws[:], wfl)
            # phased stream: column-half A (s 0-4095) of every chunk first,
            # then column-half B — pass A's whole contraction completes and
            # drains mid-stream, pass B mirrors it under the second half;
            # only pass B's final pair trails the last byte.
            xs = sb.tile([128, ND, S], dt.float8e4)
            for c in range(ND):
                nc.sync.dma_start(xs[:, c:c + 1, 0:4096],
                                  xT8[c * 128:(c + 1) * 128, 0:4096])
            for c in range(ND - 2):
                nc.sync.dma_start(xs[:, c:c + 1, 4096:8192],
                                  xT8[c * 128:(c + 1) * 128, 4096:8192])
            # last two B-halves in 1024-col pieces so each slab's final
            # matmul starts on its own slice
            for i in range(4):
                for c in (ND - 2, ND - 1):
                    nc.sync.dma_start(
                        xs[:, c:c + 1, 4096 + i * 1024:4096 + (i + 1) * 1024],
                        xT8[c * 128:(c + 1) * 128,
                            4096 + i * 1024:4096 + (i + 1) * 1024])
            stg = sb.tile([33, S], dt.bfloat16)

            banks = [ps.tile([128, 512], dt.float32, name=f"bk{i}")
                     for i in range(8)]
            # pass A rides the A-half stream
            for p in range(NP):
                for i in range(8):
                    nc.tensor.matmul(
                        banks[i][0:33, :],
                        ws[:, 2 * p:2 * p + 2, 0:33],
                        xs[:, 2 * p:2 * p + 2, i * 512:(i + 1) * 512],
                        start=(p == 0), stop=(p == NP - 1), perf_mode=DR)
            for i in range(8):
                eng = nc.vector.tensor_copy if i % 2 == 0 else nc.scalar.copy
                eng(stg[:, i * 512:(i + 1) * 512], banks[i][0:33, :])
            nc.sync.dma_start(sl[:, 0:4096], stg[:, 0:4096])
            # pass B rides the B-half stream (same banks, fresh groups)
            for p in range(NP - 1):
                for i in range(8):
                    nc.tensor.matmul(
                        banks[i][0:33, :],
                        ws[:, 2 * p:2 * p + 2, 0:33],
                        xs[:, 2 * p:2 * p + 2,
                           4096 + i * 512:4096 + (i + 1) * 512],
                        start=(p == 0), stop=False, perf_mode=DR)
            p = NP - 1
            for i in range(8):
                nc.tensor.matmul(
                    banks[i][0:33, :],
                    ws[:, 2 * p:2 * p + 2, 0:33],
                    xs[:, 2 * p:2 * p + 2,
                       4096 + i * 512:4096 + (i + 1) * 512],
                    start=False, stop=True, perf_mode=DR)
                eng = nc.vector.tensor_copy if i % 2 == 0 else nc.scalar.copy
                eng(stg[:, 4096 + i * 512:4096 + (i + 1) * 512],
                    banks[i][0:33, :])
                if i == 1:
                    nc.sync.dma_start(sl[:, 4096:5120], stg[:, 4096:5120])
                elif i == 5:
                    nc.sync.dma_start(sl[:, 5120:7168], stg[:, 5120:7168])
                elif i == 7:
                    nc.sync.dma_start(sl[:, 7168:8192], stg[:, 7168:8192])
    nc.compile()
    return nc


# ---------------------------------------------------------------- launch 2
def _build_l2():
    """ax[h, d] = sum_k attn[k, h] * x_sel[k, d] in bf16.

    attnT chunks [128k, 16] are STATIONARY (M=16), x_sel streams as the
    moving operand -> out [16, D] f32 in 4 PSUM banks; contraction over
    the 16 k-chunks overlaps the x_sel DMA stream; one row-contiguous
    output DMA."""
    nc = bacc.Bacc("TRN2", target_bir_lowering=False, debug=False,
                   num_devices=NCORES)
    NK = TOPK // 128     # 16 k-chunks
    xsel = nc.dram_tensor("xsel", [TOPK, D], dt.bfloat16,
                          kind="ExternalInput").ap()
    att = nc.dram_tensor("att", [128, NK, H], dt.bfloat16,
                         kind="ExternalInput").ap()
    axh = nc.dram_tensor("axh", [H, D], dt.float32, kind="ExternalOutput").ap()

    with tile.TileContext(nc) as tc:
        with (
            tc.tile_pool(name="sb", bufs=1) as sb,
            tc.tile_pool(name="ps", bufs=1, space="PSUM") as ps,
        ):
            ats = sb.tile([128, NK, H], dt.bfloat16)
            nc.scalar.dma_start(ats[:], att)
            xss = sb.tile([128, NK, D], dt.bfloat16)
            for kc in range(NK - 2):
                nc.sync.dma_start(xss[:, kc:kc + 1, :],
                                  xsel[kc * 128:(kc + 1) * 128, :])
            for nb in range(2):
                nc.sync.dma_start(
                    xss[:, NK - 2:NK - 1, nb * 1024:(nb + 1) * 1024],
                    xsel[(NK - 2) * 128:(NK - 1) * 128,
                         nb * 1024:(nb + 1) * 1024])
            # last k-chunk in 512-col pieces: each nb-matmul starts on its
            # own slice instead of waiting for the full chunk
            for nb in range(4):
                nc.sync.dma_start(
                    xss[:, NK - 1:NK, nb * 512:(nb + 1) * 512],
                    xsel[(NK - 1) * 128:NK * 128, nb * 512:(nb + 1) * 512])
            banks = [ps.tile([128, 512], dt.float32, name=f"ab{i}")
                     for i in range(4)]
            stg = sb.tile([H, D], dt.float32)
            for kc in range(NK - 1):
                for nb in range(4):
                    nc.tensor.matmul(
                        banks[nb][0:H, :],
                        ats[:, kc:kc + 1, :],
                        xss[:, kc:kc + 1, nb * 512:(nb + 1) * 512],
                        start=(kc == 0), stop=False)
            # final k-chunk: drain each bank right behind its last matmul
            kc = NK - 1
            for nb in range(4):
                nc.tensor.matmul(
                    banks[nb][0:H, :],
                    ats[:, kc:kc + 1, :],
                    xss[:, kc:kc + 1, nb * 512:(nb + 1) * 512],
                    start=False, stop=True)
                eng = nc.vector.tensor_copy if nb % 2 == 0 else nc.scalar.copy
                eng(stg[:, nb * 512:(nb + 1) * 512], banks[nb][0:H, :])
                if nb % 2 == 1:
                    nc.sync.dma_start(
                        axh[:, (nb - 1) * 512:(nb + 1) * 512],
                        stg[:, (nb - 1) * 512:(nb + 1) * 512])
    nc.compile()
    return nc


# ---------------------------------------------------------------- timing
def model_time(nc):
    """Cost-model (TimelineSim) estimate in ns for one core."""
    from concourse.timeline_sim import TimelineSim
    return TimelineSim(nc).simulate()


def _run_spmd_retry(nc, in_maps, cores, trace=False):
    """One retry: a previously crashed process can leave the device in a
    transient NRT_EXEC_UNIT_UNRECOVERABLE state that clears on re-run."""
    try:
        return run_bass_kernel_spmd(nc, in_maps, cores, trace=trace)
    except Exception:
        import time as _t
        _t.sleep(2.0)
        return run_bass_kernel_spmd(nc, in_maps, cores, trace=trace)


def _q8(a):
    return np.asarray(a, np.float32).astype(F8).astype(np.float32)


def kernel(**inputs):
    import jax
    import jax.numpy as jnp
    cpu = jax.devices("cpu")[0]

    x = np.ascontiguousarray(np.asarray(inputs["x"], dtype=np.float32))
    Wq = np.asarray(inputs["Wq"], dtype=np.float32)
    bq = np.asarray(inputs["bq"], dtype=np.float32)
    Wkv_down = np.asarray(inputs["Wkv_down"], dtype=np.float32)
    bkv_down = np.asarray(inputs["bkv_down"], dtype=np.float32)
    Wq_down = np.asarray(inputs["Wq_down"], dtype=np.float32)
    bq_down = np.asarray(inputs["bq_down"], dtype=np.float32)
    Wkv_up = np.asarray(inputs["Wkv_up"], dtype=np.float32)
    bkv_up = np.asarray(inputs["bkv_up"], dtype=np.float32)
    Wout = np.asarray(inputs["Wout"], dtype=np.float32)
    bout = np.asarray(inputs["bout"], dtype=np.float32)
    k = int(np.asarray(inputs["top_k"]))
    assert k == TOPK, f"kernel hardcoded for top_k={TOPK}, got {k}"

    Wdk = Wkv_down[:, :L]
    bkd = bkv_down[:L]
    Wdv = Wkv_down[:, L:]
    bvd = bkv_down[L:]
    Wk_up_h = Wkv_up[:, :D].reshape(L, H, DH)
    Wv_up_h = Wkv_up[:, D:].reshape(L, H, DH)
    bk_up_h = bkv_up[:D].reshape(H, DH)
    bv_up_h = bkv_up[D:].reshape(H, DH)

    if "l1" not in _STATE:
        _STATE["l1"] = _build_l1()
    if "l2" not in _STATE:
        _STATE["l2"] = _build_l2()

    q_last = x[:, -1, :]                                   # [B, D]
    with jax.default_device(cpu):
        # bit-exact replication of the reference's fp8 indexer query + q
        q_idx = np.asarray(
            jnp.asarray(_q8(q_last)) @ jnp.asarray(_q8(Wq_down))) \
            + _q8(bq_down)                                 # [B, L]
        q = (np.asarray(jnp.asarray(q_last) @ jnp.asarray(Wq)) + bq) \
            .reshape(B, H, DH)

    # folded per-batch vectors (host f32, exact linear algebra)
    v_lh = np.einsum('lhd,bhd->blh', Wk_up_h, q)           # [B, L, H]
    w_l = np.einsum('dl,blh->bdh', Wdk, v_lh)              # [B, D, H]
    c_l = np.einsum('l,blh->bh', bkd, v_lh) \
        + np.einsum('hd,bhd->bh', bk_up_h, q)              # [B, H]
    w_eff = q_idx @ Wdk.T                                  # [B, D]
    c_s = q_idx @ bkd                                      # [B]

    # ---------------- launch 1: fused noisy scores + logits over full S
    in1 = []
    for c in range(NCORES):
        w8 = w_l[c].astype(F8).astype(np.float32)          # [D, H]
        r8 = ((w_l[c] - w8) * RESID_SCALE).astype(F8)      # [D, H] fp8
        wfl = np.zeros((D, 48), F8)
        wfl[:, 0:16] = w8.astype(F8)
        wfl[:, 16:32] = r8
        wfl[:, 32] = w_eff[c].astype(F8)
        in1.append({
            "xT8": np.ascontiguousarray(x[c].T).astype(F8),
            "wfl": np.ascontiguousarray(
                wfl.reshape(ND, 128, 48).transpose(1, 0, 2)),
        })
    r1 = _run_spmd_retry(_STATE["l1"], in1, list(range(NCORES)))
    sl = np.stack([r1.results[c]["sl"] for c in range(NCORES)]) \
        .astype(np.float32)                                # [B, 33, S]

    # ---------------- host: exact top-k set via band rescore (bit-exact)
    sel_all = []
    logit_all = []
    with jax.default_device(cpu):
        jWdk = jnp.asarray(Wdk)
        jbkd = jnp.asarray(bkd)
        for b in range(B):
            s_noisy = sl[b, 32] + c_s[b]
            order = np.argsort(-np.maximum(s_noisy, 0.0), kind="stable")
            certain = order[:k - MARGIN]
            band = order[k - MARGIN:k + MARGIN]
            Kb = jnp.asarray(x[b][band]) @ jWdk + jbkd
            sb = np.asarray(jnp.einsum(
                "l,sl->s", jnp.asarray(q_idx[b]),
                Kb.astype(jnp.float8_e4m3fn).astype(jnp.float32)))
            sb = np.maximum(sb, 0.0)
            pick = band[np.argsort(-sb, kind="stable")[:k - len(certain)]]
            sel = np.concatenate([certain, pick])
            sel_all.append(sel)
            lg = (sl[b, 0:16][:, sel].T + sl[b, 16:32][:, sel].T / RESID_SCALE
                  + c_l[b][None, :]) * RSQ                 # [k, H]
            lg -= lg.max(axis=0, keepdims=True)
            e = np.exp(lg)
            logit_all.append(e / e.sum(axis=0, keepdims=True))

    # ---------------- launch 2: attention value aggregation
    NK = TOPK // 128
    in2 = []
    for c in range(NCORES):
        attT = logit_all[c].astype(BF16)                   # [k, H]
        in2.append({
            "xsel": x[c][sel_all[c]].astype(BF16),
            "att": np.ascontiguousarray(
                attT.reshape(NK, 128, H).transpose(1, 0, 2)),
        })
    r2 = _run_spmd_retry(_STATE["l2"], in2, list(range(NCORES)))
    ax = np.stack([r2.results[c]["axh"] for c in range(NCORES)])  # [B,H,D]

    # ---------------- host tail: tiny per-token projections (f32)
    o_lat = np.einsum('bhd,dl->bhl', ax, Wdv) + bvd        # [B, H, L]
    o = np.einsum('bhl,lhd->bhd', o_lat, Wv_up_h) + bv_up_h
    out = o.reshape(B, D) @ Wout + bout
    return out.astype(np.float32)


# revision 27
# speedup vs baseline: 1.0087x; 1.0012x over previous
"""DeepSeek sparse attention (single-query, MQA low-rank KV) on 8 trn2 cores.

Folded-indexer design (data-parallel: batch b -> core b):

The low-rank structure lets every per-token device matmul collapse into a
small set of per-batch vectors computed once on host:
  scores[s] = x[s] . w_eff          w_eff = Wdk @ q_idx           [D]
  logit[h,s] = x[s] . w_l[:,h]      w_l = Wdk @ (Wk_up_h @ q_h)   [D, 16]
so launch 1 is ONE fused fp8 DoubleRow matmul stream over x^T (the full
16.8 MB shard read once, memory-bound): stationary = [16 fp8 logit cols |
16 fp8 residual cols (64x error compensation) | 1 score col], out =
[33, 8192] f32.  Host: top-k with exact band rescore (bit-exact vs the
reference's fp8 indexer), softmax over gathered logits.
Launch 2: attention value aggregation ax[d,h] = sum_k attn[k,h]*x_sel[k,d]
(bf16, 8 MiB stream).  Host tail: tiny per-token GEMMs (ax @ Wdv etc.,
~0.5 GFLOP total, same class as the baseline's host-side q projection).

Shapes hardcoded: B=8, S=8192, D=2048, H=16, dh=128, L=512, k=2048.
"""
import numpy as np
import ml_dtypes

import concourse.bacc as bacc
import concourse.tile as tile
import concourse.mybir as mybir
from concourse.bass_utils import run_bass_kernel_spmd

BF16 = ml_dtypes.bfloat16
F8 = ml_dtypes.float8_e4m3fn
dt = mybir.dt
DR = mybir.MatmulPerfMode.DoubleRow

B, S, D = 8, 8192, 2048
H, DH, L = 16, 128, 512
TOPK = 2048
MARGIN = 768
NCORES = 8
RSQ = float(1.0 / np.sqrt(np.float32(DH)))
RESID_SCALE = 64.0

ND = D // 128            # 16 d-chunks
NP = ND // 2             # 8 DoubleRow chunk pairs

_STATE = {}


# ---------------------------------------------------------------- launch 1
def _build_l1():
    """Fused scores+logits+residuals over the full sequence.

    PSUM holds [33, 4096] f32 per pass (8 banks x [33, 512]); two passes
    cover S=8192. Pass A overlaps the x DMA stream; pass B runs from
    SBUF-resident x right after, bank-by-bank behind pass A's drains."""
    nc = bacc.Bacc("TRN2", target_bir_lowering=False, debug=False,
                   num_devices=NCORES)
    xT8 = nc.dram_tensor("xT8", [D, S], dt.float8e4, kind="ExternalInput").ap()
    wfl = nc.dram_tensor("wfl", [128, ND, 48], dt.float8e4,
                         kind="ExternalInput").ap()
    sl = nc.dram_tensor("sl", [33, S], dt.bfloat16, kind="ExternalOutput").ap()

    with tile.TileContext(nc) as tc:
        with (
            tc.tile_pool(name="sb", bufs=1) as sb,
            tc.tile_pool(name="ps", bufs=1, space="PSUM") as ps,
        ):
            ws = sb.tile([128, ND, 48], dt.float8e4)
            nc.scalar.dma_start(ws[:], wfl)
            # phased stream: column-half A (s 0-4095) of every chunk first,
            # then column-half B — pass A's whole contraction completes and
            # drains mid-stream, pass B mirrors it under the second half;
            # only pass B's final pair trails the last byte.
            xs = sb.tile([128, ND, S], dt.float8e4)
            for c in range(ND):
                nc.sync.dma_start(xs[:, c:c + 1, 0:4096],
                                  xT8[c * 128:(c + 1) * 128, 0:4096])
            for c in range(ND - 2):
                for h in range(2):
                    nc.sync.dma_start(
                        xs[:, c:c + 1, 4096 + h * 2048:4096 + (h + 1) * 2048],
                        xT8[c * 128:(c + 1) * 128,
                            4096 + h * 2048:4096 + (h + 1) * 2048])
            # last two B-halves in 1024-col pieces so each slab's final
            # matmul starts on its own slice
            for i in range(4):
                for c in (ND - 2, ND - 1):
                    nc.sync.dma_start(
                        xs[:, c:c + 1, 4096 + i * 1024:4096 + (i + 1) * 1024],
                        xT8[c * 128:(c + 1) * 128,
                            4096 + i * 1024:4096 + (i + 1) * 1024])
            stg = sb.tile([33, S], dt.bfloat16)

            banks = [ps.tile([128, 512], dt.float32, name=f"bk{i}")
                     for i in range(8)]
            # pass A rides the A-half stream
            for p in range(NP):
                for i in range(8):
                    nc.tensor.matmul(
                        banks[i][0:33, :],
                        ws[:, 2 * p:2 * p + 2, 0:33],
                        xs[:, 2 * p:2 * p + 2, i * 512:(i + 1) * 512],
                        start=(p == 0), stop=(p == NP - 1), perf_mode=DR)
            for i in range(8):
                eng = nc.vector.tensor_copy if i % 2 == 0 else nc.scalar.copy
                eng(stg[:, i * 512:(i + 1) * 512], banks[i][0:33, :])
            nc.sync.dma_start(sl[:, 0:4096], stg[:, 0:4096])
            # pass B rides the B-half stream (same banks, fresh groups)
            for p in range(NP - 1):
                for i in range(8):
                    nc.tensor.matmul(
                        banks[i][0:33, :],
                        ws[:, 2 * p:2 * p + 2, 0:33],
                        xs[:, 2 * p:2 * p + 2,
                           4096 + i * 512:4096 + (i + 1) * 512],
                        start=(p == 0), stop=False, perf_mode=DR)
            p = NP - 1
            for i in range(8):
                nc.tensor.matmul(
                    banks[i][0:33, :],
                    ws[:, 2 * p:2 * p + 2, 0:33],
                    xs[:, 2 * p:2 * p + 2,
                       4096 + i * 512:4096 + (i + 1) * 512],
                    start=False, stop=True, perf_mode=DR)
                eng = nc.vector.tensor_copy if i % 2 == 0 else nc.scalar.copy
                eng(stg[:, 4096 + i * 512:4096 + (i + 1) * 512],
                    banks[i][0:33, :])
                if i == 1:
                    nc.sync.dma_start(sl[:, 4096:5120], stg[:, 4096:5120])
                elif i == 5:
                    nc.sync.dma_start(sl[:, 5120:7168], stg[:, 5120:7168])
                elif i == 7:
                    nc.sync.dma_start(sl[:, 7168:8192], stg[:, 7168:8192])
    nc.compile()
    return nc


# ---------------------------------------------------------------- launch 2
def _build_l2():
    """ax[h, d] = sum_k attn[k, h] * x_sel[k, d] in bf16.

    attnT chunks [128k, 16] are STATIONARY (M=16), x_sel streams as the
    moving operand -> out [16, D] f32 in 4 PSUM banks; contraction over
    the 16 k-chunks overlaps the x_sel DMA stream; one row-contiguous
    output DMA."""
    nc = bacc.Bacc("TRN2", target_bir_lowering=False, debug=False,
                   num_devices=NCORES)
    NK = TOPK // 128     # 16 k-chunks
    xsel = nc.dram_tensor("xsel", [TOPK, D], dt.bfloat16,
                          kind="ExternalInput").ap()
    att = nc.dram_tensor("att", [128, NK, H], dt.bfloat16,
                         kind="ExternalInput").ap()
    axh = nc.dram_tensor("axh", [H, D], dt.float32, kind="ExternalOutput").ap()

    with tile.TileContext(nc) as tc:
        with (
            tc.tile_pool(name="sb", bufs=1) as sb,
            tc.tile_pool(name="ps", bufs=1, space="PSUM") as ps,
        ):
            ats = sb.tile([128, NK, H], dt.bfloat16)
            nc.scalar.dma_start(ats[:], att)
            xss = sb.tile([128, NK, D], dt.bfloat16)
            for kc in range(NK - 2):
                nc.sync.dma_start(xss[:, kc:kc + 1, :],
                                  xsel[kc * 128:(kc + 1) * 128, :])
            for nb in range(2):
                nc.sync.dma_start(
                    xss[:, NK - 2:NK - 1, nb * 1024:(nb + 1) * 1024],
                    xsel[(NK - 2) * 128:(NK - 1) * 128,
                         nb * 1024:(nb + 1) * 1024])
            # last k-chunk in 512-col pieces: each nb-matmul starts on its
            # own slice instead of waiting for the full chunk
            for nb in range(4):
                nc.sync.dma_start(
                    xss[:, NK - 1:NK, nb * 512:(nb + 1) * 512],
                    xsel[(NK - 1) * 128:NK * 128, nb * 512:(nb + 1) * 512])
            banks = [ps.tile([128, 512], dt.float32, name=f"ab{i}")
                     for i in range(4)]
            stg = sb.tile([H, D], dt.float32)
            for kc in range(NK - 1):
                for nb in range(4):
                    nc.tensor.matmul(
                        banks[nb][0:H, :],
                        ats[:, kc:kc + 1, :],
                        xss[:, kc:kc + 1, nb * 512:(nb + 1) * 512],
                        start=(kc == 0), stop=False)
            # final k-chunk: drain each bank right behind its last matmul
            kc = NK - 1
            for nb in range(4):
                nc.tensor.matmul(
                    banks[nb][0:H, :],
                    ats[:, kc:kc + 1, :],
                    xss[:, kc:kc + 1, nb * 512:(nb + 1) * 512],
                    start=False, stop=True)
                eng = nc.vector.tensor_copy if nb % 2 == 0 else nc.scalar.copy
                eng(stg[:, nb * 512:(nb + 1) * 512], banks[nb][0:H, :])
                if nb % 2 == 1:
                    nc.sync.dma_start(
                        axh[:, (nb - 1) * 512:(nb + 1) * 512],
                        stg[:, (nb - 1) * 512:(nb + 1) * 512])
    nc.compile()
    return nc


# ---------------------------------------------------------------- timing
def model_time(nc):
    """Cost-model (TimelineSim) estimate in ns for one core."""
    from concourse.timeline_sim import TimelineSim
    return TimelineSim(nc).simulate()


def _run_spmd_retry(nc, in_maps, cores, trace=False):
    """One retry: a previously crashed process can leave the device in a
    transient NRT_EXEC_UNIT_UNRECOVERABLE state that clears on re-run."""
    try:
        return run_bass_kernel_spmd(nc, in_maps, cores, trace=trace)
    except Exception:
        import time as _t
        _t.sleep(2.0)
        return run_bass_kernel_spmd(nc, in_maps, cores, trace=trace)


def _q8(a):
    return np.asarray(a, np.float32).astype(F8).astype(np.float32)


def kernel(**inputs):
    import jax
    import jax.numpy as jnp
    cpu = jax.devices("cpu")[0]

    x = np.ascontiguousarray(np.asarray(inputs["x"], dtype=np.float32))
    Wq = np.asarray(inputs["Wq"], dtype=np.float32)
    bq = np.asarray(inputs["bq"], dtype=np.float32)
    Wkv_down = np.asarray(inputs["Wkv_down"], dtype=np.float32)
    bkv_down = np.asarray(inputs["bkv_down"], dtype=np.float32)
    Wq_down = np.asarray(inputs["Wq_down"], dtype=np.float32)
    bq_down = np.asarray(inputs["bq_down"], dtype=np.float32)
    Wkv_up = np.asarray(inputs["Wkv_up"], dtype=np.float32)
    bkv_up = np.asarray(inputs["bkv_up"], dtype=np.float32)
    Wout = np.asarray(inputs["Wout"], dtype=np.float32)
    bout = np.asarray(inputs["bout"], dtype=np.float32)
    k = int(np.asarray(inputs["top_k"]))
    assert k == TOPK, f"kernel hardcoded for top_k={TOPK}, got {k}"

    Wdk = Wkv_down[:, :L]
    bkd = bkv_down[:L]
    Wdv = Wkv_down[:, L:]
    bvd = bkv_down[L:]
    Wk_up_h = Wkv_up[:, :D].reshape(L, H, DH)
    Wv_up_h = Wkv_up[:, D:].reshape(L, H, DH)
    bk_up_h = bkv_up[:D].reshape(H, DH)
    bv_up_h = bkv_up[D:].reshape(H, DH)

    if "l1" not in _STATE:
        _STATE["l1"] = _build_l1()
    if "l2" not in _STATE:
        _STATE["l2"] = _build_l2()

    q_last = x[:, -1, :]                                   # [B, D]
    with jax.default_device(cpu):
        # bit-exact replication of the reference's fp8 indexer query + q
        q_idx = np.asarray(
            jnp.asarray(_q8(q_last)) @ jnp.asarray(_q8(Wq_down))) \
            + _q8(bq_down)                                 # [B, L]
        q = (np.asarray(jnp.asarray(q_last) @ jnp.asarray(Wq)) + bq) \
            .reshape(B, H, DH)

    # folded per-batch vectors (host f32, exact linear algebra)
    v_lh = np.einsum('lhd,bhd->blh', Wk_up_h, q)           # [B, L, H]
    w_l = np.einsum('dl,blh->bdh', Wdk, v_lh)              # [B, D, H]
    c_l = np.einsum('l,blh->bh', bkd, v_lh) \
        + np.einsum('hd,bhd->bh', bk_up_h, q)              # [B, H]
    w_eff = q_idx @ Wdk.T                                  # [B, D]
    c_s = q_idx @ bkd                                      # [B]

    # ---------------- launch 1: fused noisy scores + logits over full S
    in1 = []
    for c in range(NCORES):
        w8 = w_l[c].astype(F8).astype(np.float32)          # [D, H]
        r8 = ((w_l[c] - w8) * RESID_SCALE).astype(F8)      # [D, H] fp8
        wfl = np.zeros((D, 48), F8)
        wfl[:, 0:16] = w8.astype(F8)
        wfl[:, 16:32] = r8
        wfl[:, 32] = w_eff[c].astype(F8)
        in1.append({
            "xT8": np.ascontiguousarray(x[c].T).astype(F8),
            "wfl": np.ascontiguousarray(
                wfl.reshape(ND, 128, 48).transpose(1, 0, 2)),
        })
    r1 = _run_spmd_retry(_STATE["l1"], in1, list(range(NCORES)))
    sl = np.stack([r1.results[c]["sl"] for c in range(NCORES)]) \
        .astype(np.float32)                                # [B, 33, S]

    # ---------------- host: exact top-k set via band rescore (bit-exact)
    sel_all = []
    logit_all = []
    with jax.default_device(cpu):
        jWdk = jnp.asarray(Wdk)
        jbkd = jnp.asarray(bkd)
        for b in range(B):
            s_noisy = sl[b, 32] + c_s[b]
            order = np.argsort(-np.maximum(s_noisy, 0.0), kind="stable")
            certain = order[:k - MARGIN]
            band = order[k - MARGIN:k + MARGIN]
            Kb = jnp.asarray(x[b][band]) @ jWdk + jbkd
            sb = np.asarray(jnp.einsum(
                "l,sl->s", jnp.asarray(q_idx[b]),
                Kb.astype(jnp.float8_e4m3fn).astype(jnp.float32)))
            sb = np.maximum(sb, 0.0)
            pick = band[np.argsort(-sb, kind="stable")[:k - len(certain)]]
            sel = np.concatenate([certain, pick])
            sel_all.append(sel)
            lg = (sl[b, 0:16][:, sel].T + sl[b, 16:32][:, sel].T / RESID_SCALE
                  + c_l[b][None, :]) * RSQ                 # [k, H]
            lg -= lg.max(axis=0, keepdims=True)
            e = np.exp(lg)
            logit_all.append(e / e.sum(axis=0, keepdims=True))

    # ---------------- launch 2: attention value aggregation
    NK = TOPK // 128
    in2 = []
    for c in range(NCORES):
        attT = logit_all[c].astype(BF16)                   # [k, H]
        in2.append({
            "xsel": x[c][sel_all[c]].astype(BF16),
            "att": np.ascontiguousarray(
                attT.reshape(NK, 128, H).transpose(1, 0, 2)),
        })
    r2 = _run_spmd_retry(_STATE["l2"], in2, list(range(NCORES)))
    ax = np.stack([r2.results[c]["axh"] for c in range(NCORES)])  # [B,H,D]

    # ---------------- host tail: tiny per-token projections (f32)
    o_lat = np.einsum('bhd,dl->bhl', ax, Wdv) + bvd        # [B, H, L]
    o = np.einsum('bhl,lhd->bhd', o_lat, Wv_up_h) + bv_up_h
    out = o.reshape(B, D) @ Wout + bout
    return out.astype(np.float32)


# revision 28
# speedup vs baseline: 1.0093x; 1.0006x over previous
"""DeepSeek sparse attention (single-query, MQA low-rank KV) on 8 trn2 cores.

Folded-indexer design (data-parallel: batch b -> core b):

The low-rank structure lets every per-token device matmul collapse into a
small set of per-batch vectors computed once on host:
  scores[s] = x[s] . w_eff          w_eff = Wdk @ q_idx           [D]
  logit[h,s] = x[s] . w_l[:,h]      w_l = Wdk @ (Wk_up_h @ q_h)   [D, 16]
so launch 1 is ONE fused fp8 DoubleRow matmul stream over x^T (the full
16.8 MB shard read once, memory-bound): stationary = [16 fp8 logit cols |
16 fp8 residual cols (64x error compensation) | 1 score col], out =
[33, 8192] f32.  Host: top-k with exact band rescore (bit-exact vs the
reference's fp8 indexer), softmax over gathered logits.
Launch 2: attention value aggregation ax[d,h] = sum_k attn[k,h]*x_sel[k,d]
(bf16, 8 MiB stream).  Host tail: tiny per-token GEMMs (ax @ Wdv etc.,
~0.5 GFLOP total, same class as the baseline's host-side q projection).

Shapes hardcoded: B=8, S=8192, D=2048, H=16, dh=128, L=512, k=2048.
"""
import numpy as np
import ml_dtypes

import concourse.bacc as bacc
import concourse.tile as tile
import concourse.mybir as mybir
from concourse.bass_utils import run_bass_kernel_spmd

BF16 = ml_dtypes.bfloat16
F8 = ml_dtypes.float8_e4m3fn
dt = mybir.dt
DR = mybir.MatmulPerfMode.DoubleRow

B, S, D = 8, 8192, 2048
H, DH, L = 16, 128, 512
TOPK = 2048
MARGIN = 768
NCORES = 8
RSQ = float(1.0 / np.sqrt(np.float32(DH)))
RESID_SCALE = 64.0

ND = D // 128            # 16 d-chunks
NP = ND // 2             # 8 DoubleRow chunk pairs

_STATE = {}


# ---------------------------------------------------------------- launch 1
def _build_l1():
    """Fused scores+logits+residuals over the full sequence.

    PSUM holds [33, 4096] f32 per pass (8 banks x [33, 512]); two passes
    cover S=8192. Pass A overlaps the x DMA stream; pass B runs from
    SBUF-resident x right after, bank-by-bank behind pass A's drains."""
    nc = bacc.Bacc("TRN2", target_bir_lowering=False, debug=False,
                   num_devices=NCORES)
    xT8 = nc.dram_tensor("xT8", [D, S], dt.float8e4, kind="ExternalInput").ap()
    wfl = nc.dram_tensor("wfl", [128, ND, 48], dt.float8e4,
                         kind="ExternalInput").ap()
    sl = nc.dram_tensor("sl", [33, S], dt.bfloat16, kind="ExternalOutput").ap()

    with tile.TileContext(nc) as tc:
        with (
            tc.tile_pool(name="sb", bufs=1) as sb,
            tc.tile_pool(name="ps", bufs=1, space="PSUM") as ps,
        ):
            ws = sb.tile([128, ND, 48], dt.float8e4)
            nc.scalar.dma_start(ws[:], wfl)
            # phased stream: column-half A (s 0-4095) of every chunk first,
            # then column-half B — pass A's whole contraction completes and
            # drains mid-stream, pass B mirrors it under the second half;
            # only pass B's final pair trails the last byte.
            xs = sb.tile([128, ND, S], dt.float8e4)
            for c in range(ND):
                nc.sync.dma_start(xs[:, c:c + 1, 0:4096],
                                  xT8[c * 128:(c + 1) * 128, 0:4096])
            for c in range(ND - 2):
                for h in range(2):
                    nc.sync.dma_start(
                        xs[:, c:c + 1, 4096 + h * 2048:4096 + (h + 1) * 2048],
                        xT8[c * 128:(c + 1) * 128,
                            4096 + h * 2048:4096 + (h + 1) * 2048])
            # last two B-halves in 1024-col pieces so each slab's final
            # matmul starts on its own slice
            for i in range(4):
                for c in (ND - 2, ND - 1):
                    nc.sync.dma_start(
                        xs[:, c:c + 1, 4096 + i * 1024:4096 + (i + 1) * 1024],
                        xT8[c * 128:(c + 1) * 128,
                            4096 + i * 1024:4096 + (i + 1) * 1024])
            stg = sb.tile([33, S], dt.bfloat16)

            banks = [ps.tile([128, 512], dt.float32, name=f"bk{i}")
                     for i in range(8)]
            # pass A rides the A-half stream
            for p in range(NP):
                for i in range(8):
                    nc.tensor.matmul(
                        banks[i][0:33, :],
                        ws[:, 2 * p:2 * p + 2, 0:33],
                        xs[:, 2 * p:2 * p + 2, i * 512:(i + 1) * 512],
                        start=(p == 0), stop=(p == NP - 1), perf_mode=DR)
            for i in range(8):
                eng = nc.vector.tensor_copy if i % 2 == 0 else nc.scalar.copy
                eng(stg[:, i * 512:(i + 1) * 512], banks[i][0:33, :])
            nc.sync.dma_start(sl[:, 0:4096], stg[:, 0:4096])
            # pass B rides the B-half stream (same banks, fresh groups)
            for p in range(NP - 1):
                for i in range(8):
                    nc.tensor.matmul(
                        banks[i][0:33, :],
                        ws[:, 2 * p:2 * p + 2, 0:33],
                        xs[:, 2 * p:2 * p + 2,
                           4096 + i * 512:4096 + (i + 1) * 512],
                        start=(p == 0), stop=False, perf_mode=DR)
            p = NP - 1
            for i in range(8):
                nc.tensor.matmul(
                    banks[i][0:33, :],
                    ws[:, 2 * p:2 * p + 2, 0:33],
                    xs[:, 2 * p:2 * p + 2,
                       4096 + i * 512:4096 + (i + 1) * 512],
                    start=False, stop=True, perf_mode=DR)
                eng = nc.vector.tensor_copy if i % 2 == 0 else nc.scalar.copy
                eng(stg[:, 4096 + i * 512:4096 + (i + 1) * 512],
                    banks[i][0:33, :])
                if i == 1:
                    nc.sync.dma_start(sl[:, 4096:5120], stg[:, 4096:5120])
                elif i == 5:
                    nc.sync.dma_start(sl[:, 5120:7168], stg[:, 5120:7168])
                elif i == 7:
                    nc.sync.dma_start(sl[:, 7168:8192], stg[:, 7168:8192])
    nc.compile()
    return nc


# ---------------------------------------------------------------- launch 2
def _build_l2():
    """ax[h, d] = sum_k attn[k, h] * x_sel[k, d] in bf16.

    attnT chunks [128k, 16] are STATIONARY (M=16), x_sel streams as the
    moving operand -> out [16, D] f32 in 4 PSUM banks; contraction over
    the 16 k-chunks overlaps the x_sel DMA stream; one row-contiguous
    output DMA."""
    nc = bacc.Bacc("TRN2", target_bir_lowering=False, debug=False,
                   num_devices=NCORES)
    NK = TOPK // 128     # 16 k-chunks
    xsel = nc.dram_tensor("xsel", [TOPK, D], dt.bfloat16,
                          kind="ExternalInput").ap()
    att = nc.dram_tensor("att", [128, NK, H], dt.bfloat16,
                         kind="ExternalInput").ap()
    axh = nc.dram_tensor("axh", [H, D], dt.float32, kind="ExternalOutput").ap()

    with tile.TileContext(nc) as tc:
        with (
            tc.tile_pool(name="sb", bufs=1) as sb,
            tc.tile_pool(name="ps", bufs=1, space="PSUM") as ps,
        ):
            ats = sb.tile([128, NK, H], dt.bfloat16)
            nc.scalar.dma_start(ats[:], att)
            xss = sb.tile([128, NK, D], dt.bfloat16)
            for kc in range(NK - 3):
                nc.sync.dma_start(xss[:, kc:kc + 1, :],
                                  xsel[kc * 128:(kc + 1) * 128, :])
            for h in range(2):
                nc.sync.dma_start(
                    xss[:, NK - 3:NK - 2, h * 1024:(h + 1) * 1024],
                    xsel[(NK - 3) * 128:(NK - 2) * 128,
                         h * 1024:(h + 1) * 1024])
            for nb in range(2):
                nc.sync.dma_start(
                    xss[:, NK - 2:NK - 1, nb * 1024:(nb + 1) * 1024],
                    xsel[(NK - 2) * 128:(NK - 1) * 128,
                         nb * 1024:(nb + 1) * 1024])
            # last k-chunk in 512-col pieces: each nb-matmul starts on its
            # own slice instead of waiting for the full chunk
            for nb in range(4):
                nc.sync.dma_start(
                    xss[:, NK - 1:NK, nb * 512:(nb + 1) * 512],
                    xsel[(NK - 1) * 128:NK * 128, nb * 512:(nb + 1) * 512])
            banks = [ps.tile([128, 512], dt.float32, name=f"ab{i}")
                     for i in range(4)]
            stg = sb.tile([H, D], dt.float32)
            for kc in range(NK - 1):
                for nb in range(4):
                    nc.tensor.matmul(
                        banks[nb][0:H, :],
                        ats[:, kc:kc + 1, :],
                        xss[:, kc:kc + 1, nb * 512:(nb + 1) * 512],
                        start=(kc == 0), stop=False)
            # final k-chunk: drain each bank right behind its last matmul
            kc = NK - 1
            for nb in range(4):
                nc.tensor.matmul(
                    banks[nb][0:H, :],
                    ats[:, kc:kc + 1, :],
                    xss[:, kc:kc + 1, nb * 512:(nb + 1) * 512],
                    start=False, stop=True)
                eng = nc.vector.tensor_copy if nb % 2 == 0 else nc.scalar.copy
                eng(stg[:, nb * 512:(nb + 1) * 512], banks[nb][0:H, :])
                if nb % 2 == 1:
                    nc.sync.dma_start(
                        axh[:, (nb - 1) * 512:(nb + 1) * 512],
                        stg[:, (nb - 1) * 512:(nb + 1) * 512])
    nc.compile()
    return nc


# ---------------------------------------------------------------- timing
def model_time(nc):
    """Cost-model (TimelineSim) estimate in ns for one core."""
    from concourse.timeline_sim import TimelineSim
    return TimelineSim(nc).simulate()


def _run_spmd_retry(nc, in_maps, cores, trace=False):
    """One retry: a previously crashed process can leave the device in a
    transient NRT_EXEC_UNIT_UNRECOVERABLE state that clears on re-run."""
    try:
        return run_bass_kernel_spmd(nc, in_maps, cores, trace=trace)
    except Exception:
        import time as _t
        _t.sleep(2.0)
        return run_bass_kernel_spmd(nc, in_maps, cores, trace=trace)


def _q8(a):
    return np.asarray(a, np.float32).astype(F8).astype(np.float32)


def kernel(**inputs):
    import jax
    import jax.numpy as jnp
    cpu = jax.devices("cpu")[0]

    x = np.ascontiguousarray(np.asarray(inputs["x"], dtype=np.float32))
    Wq = np.asarray(inputs["Wq"], dtype=np.float32)
    bq = np.asarray(inputs["bq"], dtype=np.float32)
    Wkv_down = np.asarray(inputs["Wkv_down"], dtype=np.float32)
    bkv_down = np.asarray(inputs["bkv_down"], dtype=np.float32)
    Wq_down = np.asarray(inputs["Wq_down"], dtype=np.float32)
    bq_down = np.asarray(inputs["bq_down"], dtype=np.float32)
    Wkv_up = np.asarray(inputs["Wkv_up"], dtype=np.float32)
    bkv_up = np.asarray(inputs["bkv_up"], dtype=np.float32)
    Wout = np.asarray(inputs["Wout"], dtype=np.float32)
    bout = np.asarray(inputs["bout"], dtype=np.float32)
    k = int(np.asarray(inputs["top_k"]))
    assert k == TOPK, f"kernel hardcoded for top_k={TOPK}, got {k}"

    Wdk = Wkv_down[:, :L]
    bkd = bkv_down[:L]
    Wdv = Wkv_down[:, L:]
    bvd = bkv_down[L:]
    Wk_up_h = Wkv_up[:, :D].reshape(L, H, DH)
    Wv_up_h = Wkv_up[:, D:].reshape(L, H, DH)
    bk_up_h = bkv_up[:D].reshape(H, DH)
    bv_up_h = bkv_up[D:].reshape(H, DH)

    if "l1" not in _STATE:
        _STATE["l1"] = _build_l1()
    if "l2" not in _STATE:
        _STATE["l2"] = _build_l2()

    q_last = x[:, -1, :]                                   # [B, D]
    with jax.default_device(cpu):
        # bit-exact replication of the reference's fp8 indexer query + q
        q_idx = np.asarray(
            jnp.asarray(_q8(q_last)) @ jnp.asarray(_q8(Wq_down))) \
            + _q8(bq_down)                                 # [B, L]
        q = (np.asarray(jnp.asarray(q_last) @ jnp.asarray(Wq)) + bq) \
            .reshape(B, H, DH)

    # folded per-batch vectors (host f32, exact linear algebra)
    v_lh = np.einsum('lhd,bhd->blh', Wk_up_h, q)           # [B, L, H]
    w_l = np.einsum('dl,blh->bdh', Wdk, v_lh)              # [B, D, H]
    c_l = np.einsum('l,blh->bh', bkd, v_lh) \
        + np.einsum('hd,bhd->bh', bk_up_h, q)              # [B, H]
    w_eff = q_idx @ Wdk.T                                  # [B, D]
    c_s = q_idx @ bkd                                      # [B]

    # ---------------- launch 1: fused noisy scores + logits over full S
    in1 = []
    for c in range(NCORES):
        w8 = w_l[c].astype(F8).astype(np.float32)          # [D, H]
        r8 = ((w_l[c] - w8) * RESID_SCALE).astype(F8)      # [D, H] fp8
        wfl = np.zeros((D, 48), F8)
        wfl[:, 0:16] = w8.astype(F8)
        wfl[:, 16:32] = r8
        wfl[:, 32] = w_eff[c].astype(F8)
        in1.append({
            "xT8": np.ascontiguousarray(x[c].T).astype(F8),
            "wfl": np.ascontiguousarray(
                wfl.reshape(ND, 128, 48).transpose(1, 0, 2)),
        })
    r1 = _run_spmd_retry(_STATE["l1"], in1, list(range(NCORES)))
    sl = np.stack([r1.results[c]["sl"] for c in range(NCORES)]) \
        .astype(np.float32)                                # [B, 33, S]

    # ---------------- host: exact top-k set via band rescore (bit-exact)
    sel_all = []
    logit_all = []
    with jax.default_device(cpu):
        jWdk = jnp.asarray(Wdk)
        jbkd = jnp.asarray(bkd)
        for b in range(B):
            s_noisy = sl[b, 32] + c_s[b]
            order = np.argsort(-np.maximum(s_noisy, 0.0), kind="stable")
            certain = order[:k - MARGIN]
            band = order[k - MARGIN:k + MARGIN]
            Kb = jnp.asarray(x[b][band]) @ jWdk + jbkd
            sb = np.asarray(jnp.einsum(
                "l,sl->s", jnp.asarray(q_idx[b]),
                Kb.astype(jnp.float8_e4m3fn).astype(jnp.float32)))
            sb = np.maximum(sb, 0.0)
            pick = band[np.argsort(-sb, kind="stable")[:k - len(certain)]]
            sel = np.concatenate([certain, pick])
            sel_all.append(sel)
            lg = (sl[b, 0:16][:, sel].T + sl[b, 16:32][:, sel].T / RESID_SCALE
                  + c_l[b][None, :]) * RSQ                 # [k, H]
            lg -= lg.max(axis=0, keepdims=True)
            e = np.exp(lg)
            logit_all.append(e / e.sum(axis=0, keepdims=True))

    # ---------------- launch 2: attention value aggregation
    NK = TOPK // 128
    in2 = []
    for c in range(NCORES):
        attT = logit_all[c].astype(BF16)                   # [k, H]
        in2.append({
            "xsel": x[c][sel_all[c]].astype(BF16),
            "att": np.ascontiguousarray(
                attT.reshape(NK, 128, H).transpose(1, 0, 2)),
        })
    r2 = _run_spmd_retry(_STATE["l2"], in2, list(range(NCORES)))
    ax = np.stack([r2.results[c]["axh"] for c in range(NCORES)])  # [B,H,D]

    # ---------------- host tail: tiny per-token projections (f32)
    o_lat = np.einsum('bhd,dl->bhl', ax, Wdv) + bvd        # [B, H, L]
    o = np.einsum('bhl,lhd->bhd', o_lat, Wv_up_h) + bv_up_h
    out = o.reshape(B, D) @ Wout + bout
    return out.astype(np.float32)
